# revision 13
# baseline (speedup 1.0000x reference)
"""Trainium2 Bass kernel for nn_CrossAttention (B=4, S=1024, C=1024, H=16).

Sharding: 8 cores = (batch b in 0..4) x (query-half qs in 0..2). Each core
computes, for its 512 query rows of batch b: the Q projection, K/V
projections over the *packed valid* key positions, masked-softmax attention
over all 16 heads, the output projection, and the MLP with residual. No
collectives.

Sparsity: the mask is per (batch, key-position) and zeroes out ~25% of key
columns entirely, so the host packs K/V to the valid positions only
(padded to a multiple of 128, VP columns). Padding columns get maskrow
-30000 -> exp==0, and zero V -> no contribution.

Bias folding: softmax over kpos makes any per-query constant cancel, so
the K-side bias bk never matters; logits == (q0+bq).k0 + mask (mod
constants). bq is added during the Q-projection psum eviction
(tensor_scalar), bv is folded into bp_eff = bp + bv@Wp on the host.

DMA layout: wq/wk are host-repacked into per-m column blocks
[m][p][k][j] so each block is one dense 2KB-per-partition transfer, and
loads are spread over four engine queues (sync/scalar/vector/gpsimd)
with the tiny constants first so nothing queues behind megabyte loads.

All activations flow transposed (contraction dim on SBUF partitions):
  qT = Wq'^T @ queryT + bq'      [c_out, q]   (Wq' = Wq*scale)
  kT = Wk^T  @ keyT_packed       [c_out, vp]
  v  = valueT_packed^T @ Wv      [vp, c_out]
  per head h with augmented rows: qTaug = [qT_h ; 1], kTaug = [kT_h ; mask]
  LTh = kTaug^T @ qTaug          [vp, q] logits^T;  PTh = exp(LTh)
  oTh = [v_h | 1]^T @ PTh        [65, q]; row 64 = softmax denominator
  xT[h] = oTh[0:64] * bcast(1/denom)
  xpT = Wp^T @ xT + bp_eff;  h1T = gelu(W1^T @ xpT + b1)
  outT = xpT + W2^T @ h1T + b2
"""

from contextlib import ExitStack

import numpy as np

import concourse.bass as bass
import concourse.tile as tile
from concourse import bacc, mybir
from concourse.bass_utils import run_bass_kernel_spmd

B, S, C, H = 4, 1024, 1024, 16
HD = C // H          # 64
SCALE = HD ** -0.5
P = 128              # SBUF partitions
SQ = S // 2          # 512 query rows per core
NCORES = 8
KT = C // P          # 8 contraction tiles of 128
N512 = 512
MASK_NEG = -30000.0

F32 = mybir.dt.float32
BF16 = mybir.dt.bfloat16
NPBF16 = mybir.dt.np(BF16)


def build_program(vp):
    ktv = vp // P    # packed key-position tiles
    nc = bacc.Bacc(None, target_bir_lowering=False, debug=False)

    # wq/wk repacked on host: [m, p, k, j] = W[k*128+p, m*128+j]
    wq = nc.dram_tensor("wq", [KT, P, KT, P], BF16, kind="ExternalInput")
    wk = nc.dram_tensor("wk", [KT, P, KT, P], BF16, kind="ExternalInput")
    wv = nc.dram_tensor("wv", [C, C], BF16, kind="ExternalInput")
    wp = nc.dram_tensor("wp", [C, C], BF16, kind="ExternalInput")
    w1 = nc.dram_tensor("w1", [C, C], BF16, kind="ExternalInput")
    w2 = nc.dram_tensor("w2", [C, C], BF16, kind="ExternalInput")
    qt_in = nc.dram_tensor("qt_in", [C, SQ], BF16, kind="ExternalInput")
    kt_in = nc.dram_tensor("kt_in", [C, vp], BF16, kind="ExternalInput")
    vt_in = nc.dram_tensor("vt_in", [C, vp], BF16, kind="ExternalInput")
    # per-channel vectors packed to [P, 4, KT]: i=0..3 -> bp_eff, b1, b2, bq'
    bvecs = nc.dram_tensor("bvecs", [P, 4, KT], F32, kind="ExternalInput")
    maskrow = nc.dram_tensor("maskrow", [vp], BF16, kind="ExternalInput")
    out = nc.dram_tensor("out", [C, SQ], F32, kind="ExternalOutput")

    add = mybir.AluOpType.add
    Act = mybir.ActivationFunctionType

    with tile.TileContext(nc) as tc, ExitStack() as ctx:
        const = ctx.enter_context(tc.tile_pool(name="const", bufs=1))
        wcol = ctx.enter_context(tc.tile_pool(name="wcol", bufs=1))
        wfull = ctx.enter_context(tc.tile_pool(name="wfull", bufs=3))
        acts = ctx.enter_context(tc.tile_pool(name="acts", bufs=1))
        ptp = ctx.enter_context(tc.tile_pool(name="ptp", bufs=4))
        smal = ctx.enter_context(tc.tile_pool(name="smal", bufs=2))
        outp = ctx.enter_context(tc.tile_pool(name="outp", bufs=3))
        ps = ctx.enter_context(tc.tile_pool(name="ps", bufs=2, space="PSUM"))
        pslt = ctx.enter_context(tc.tile_pool(name="pslt", bufs=2, space="PSUM"))
        pvp = ctx.enter_context(tc.tile_pool(name="pvp", bufs=2, space="PSUM"))

        # ---- tiles ----
        qin = acts.tile([P, KT, SQ], BF16, tag="qin_xT")
        kin = acts.tile([P, KT, vp], BF16, tag="kin_h1T")
        vin = acts.tile([P, KT, vp], BF16, tag="vin_xpT")
        bv_sb = const.tile([P, 4, KT], F32, tag="bvecs")
        bp_sb = bv_sb[:, 0, :]
        b1_sb = bv_sb[:, 1, :]
        b2_sb = bv_sb[:, 2, :]
        bq_sb = bv_sb[:, 3, :]
        qTa = acts.tile([HD + 1, H, SQ], BF16, tag="qTa")
        kTa = acts.tile([HD + 1, H, vp], BF16, tag="kTa")
        vaug = acts.tile([P, ktv, H * 65], BF16, tag="va")  # [kpos, h*(64|1)]
        xT = acts.tile([P, KT, SQ], BF16, tag="qin_xT")        # [c, q] attn out
        xpT = acts.tile([P, KT, SQ], BF16, tag="vin_xpT")      # [c', q] proj out
        h1T = acts.tile([P, KT, SQ], BF16, tag="kin_h1T")      # [c_h, q] hidden

        # ---- DMA queues: constants first on vector; inputs split over
        # sync+scalar; wq on gpsimd; wk on sync after the inputs ----
        nc.scalar.dma_start(bv_sb[:, :, :], bvecs[:, :, :])
        mr = maskrow.ap()
        mrow_bcast = bass.AP(tensor=mr.tensor, offset=mr.offset,
                             ap=[[0, H]] + list(mr.ap))
        nc.scalar.dma_start(kTa[HD:HD + 1, :, :], mrow_bcast)

        qin_sr = qt_in.rearrange("(k p) n -> p k n", p=P)
        nc.sync.dma_start(qin[:, 0:4, :], qin_sr[:, 0:4, :])
        nc.scalar.dma_start(qin[:, 4:8, :], qin_sr[:, 4:8, :])

        def load_w_cols(w_dram, eng):
            tiles = []
            for m in range(KT):
                wm = wcol.tile([P, KT, P], BF16, tag=f"wc{m}")
                eng.dma_start(wm[:, :, :], w_dram[m, :, :, :])
                tiles.append(wm)
            return tiles

        def load_w_full(w_dram, eng):
            wsb = wfull.tile([P, KT, C], BF16, tag="w")
            eng.dma_start(wsb[:, :, :], w_dram.rearrange("(k p) n -> p k n", p=P))
            return wsb

        wqm = load_w_cols(wq, nc.gpsimd)

        kin_sr = kt_in.rearrange("(k p) n -> p k n", p=P)
        nc.sync.dma_start(kin[:, 0:4, :], kin_sr[:, 0:4, :])
        nc.scalar.dma_start(kin[:, 4:8, :], kin_sr[:, 4:8, :])

        # wfull rotation (bufs=3): wv, wp, w1, then w2 reuses wv's buffer
        # once the V projection has consumed it (~halfway in); its sync
        # issue therefore blocks only the late output DMAs.
        wv_sb = load_w_full(wv, nc.gpsimd)
        wp_sb = load_w_full(wp, nc.gpsimd)
        w1_sb = load_w_full(w1, nc.gpsimd)

        nc.vector.memset(qTa[HD:HD + 1, :, :], 1.0)        # ones row (64)
        vaug_h = vaug.rearrange("p k (h e) -> p k h e", e=65)
        for kt in range(ktv):
            nc.vector.memset(vaug_h[:, kt, :, 64:65], 1.0)

        # ---- Q projection (transposed output, + bq during eviction) ----
        for m in range(KT):
            pt = ps.tile([P, N512], F32, tag="mm")
            for k in range(KT):
                nc.tensor.matmul(
                    pt[:, :], wqm[m][:, k, :], qin[:, k, :],
                    start=(k == 0), stop=(k == KT - 1),
                )
            nc.vector.tensor_scalar(
                out=qTa[0:HD, 2 * m, :], in0=pt[0:HD, :],
                scalar1=bq_sb[0:HD, m:m + 1], scalar2=None, op0=add,
            )
            nc.vector.tensor_scalar(
                out=qTa[0:HD, 2 * m + 1, :], in0=pt[HD:P, :],
                scalar1=bq_sb[HD:P, m:m + 1], scalar2=None, op0=add,
            )

        # wk halves on sync+scalar (ahead of vin: K proj needs wk at ~26us,
        # V proj needs vin only at ~45us)
        wkm = []
        for m in range(KT):
            wm = wcol.tile([P, KT, P], BF16, tag=f"wc{m}", name=f"wkm{m}")
            (nc.sync if m < 4 else nc.scalar).dma_start(wm[:, :, :], wk[m, :, :, :])
            wkm.append(wm)
        vin_sr = vt_in.rearrange("(k p) n -> p k n", p=P)
        nc.sync.dma_start(vin[:, 0:4, :], vin_sr[:, 0:4, :])
        nc.scalar.dma_start(vin[:, 4:8, :], vin_sr[:, 4:8, :])
        w2_sb = load_w_full(w2, nc.sync)

        # n-chunks of the packed key axis (512 then the remainder)
        nchunks = []
        n0 = 0
        while n0 < vp:
            nsz = min(N512, vp - n0)
            nchunks.append((n0, nsz))
            n0 += nsz

        pTts = {}

        def emit_qk(h):
            pTt = ptp.tile([P, ktv, N512], BF16, tag="pt")
            pTts[h] = pTt
            for t in range((ktv + 1) // 2):
                lt = pslt.tile([P, 2 * N512], F32, tag="lt")
                kts = [kt for kt in (2 * t, 2 * t + 1) if kt < ktv]
                for kt in kts:
                    nc.tensor.matmul(
                        lt[:, (kt % 2) * N512:((kt % 2) + 1) * N512],
                        kTa[0:HD + 1, h, kt * P:(kt + 1) * P],
                        qTa[0:HD + 1, h, :],
                        start=True, stop=True,
                    )
                nc.scalar.activation(
                    out=pTt[:, 2 * t:2 * t + len(kts), :],
                    in_=lt[:, 0:len(kts) * N512].rearrange(
                        "p (t n) -> p t n", n=N512),
                    func=Act.Exp,
                )

        # ---- K projection; QK for heads 0-2 interleaved so the scalar
        # engine's exp stream starts early ----
        for m in range(KT):
            for n0, nsz in nchunks:
                pt = ps.tile([P, N512], F32, tag="mm")
                for k in range(KT):
                    nc.tensor.matmul(
                        pt[:, 0:nsz],
                        wkm[m][:, k, :],
                        kin[:, k, n0:n0 + nsz],
                        start=(k == 0), stop=(k == KT - 1),
                    )
                ns = slice(n0, n0 + nsz)
                nc.vector.tensor_copy(kTa[0:HD, 2 * m, ns], pt[0:HD, 0:nsz])
                nc.vector.tensor_copy(kTa[0:HD, 2 * m + 1, ns], pt[HD:P, 0:nsz])
            if m < 3:
                emit_qk(m)

        # ---- attention: V projection + remaining QK + PV pipeline ----
        def emit_v_chunk(m, n):
            pt = ps.tile([P, N512], F32, tag="mm")
            for k in range(KT):
                nc.tensor.matmul(
                    pt[:, :],
                    vin[:, k, m * P:(m + 1) * P],
                    wv_sb[:, k, n * N512:(n + 1) * N512],
                    start=(k == 0), stop=(k == KT - 1),
                )
            nc.vector.tensor_copy(
                vaug_h[:, m, 8 * n:8 * n + 8, 0:64],
                pt[:, :].rearrange("p (h d) -> p h d", d=HD),
            )

        def emit_pv(h):
            hp = (h % 2) * HD
            hm = h // 2
            pTt = pTts.pop(h)
            pv = pvp.tile([HD + 1, N512], F32, tag="pv")
            for kt in range(ktv):
                nc.tensor.matmul(
                    pv[:, :],
                    vaug[:, kt, h * 65:(h + 1) * 65],
                    pTt[:, kt, :],
                    start=(kt == 0), stop=(kt == ktv - 1),
                )
            rc = smal.tile([1, N512], F32, tag="rc")
            bc = smal.tile([HD, N512], F32, tag="bc")
            # denom >= exp(min valid logit) > 0 (each batch has >=1 valid
            # key position), so the fast reciprocal's denorm/zero edge
            # cases cannot occur
            nc.vector.tensor_copy(rc[0:1, :], pv[HD:HD + 1, :])
            nc.vector.reciprocal_approx_fast(out=rc[0:1, :], in_=rc[0:1, :])
            nc.gpsimd.partition_broadcast(bc[:, :], rc[0:1, :])
            nc.vector.tensor_mul(xT[hp:hp + HD, hm, :], pv[0:HD, :], bc[:, :])

        # V chunks n=0 first (PV_0..7 read the n=0 half of vaug), then the
        # PV pipeline with the remaining QKs and V chunks interleaved.
        # pTt window stays <= 4 tiles. The last PV slots are filled with
        # the head-independent part of the first two P-proj chains.
        for m in range(ktv):
            emit_v_chunk(m, 0)
            if m == 0:
                emit_qk(3)

        ppA = {}

        def emit_pproj_A(m):
            # k=0..6 of P-proj chain m: independent of the last PV evicts
            pt = ps.tile([P, N512], F32, tag="mm")
            ppA[m] = pt
            for k in range(KT - 1):
                nc.tensor.matmul(
                    pt[:, :], wp_sb[:, k, m * P:(m + 1) * P], xT[:, k, :],
                    start=(k == 0), stop=(k == KT - 2),
                )

        def emit_pproj_B(m):
            # k=7 into a spare pslt half; A evicts (+bias) then B adds in
            # (a vector op may read only one PSUM operand)
            bt = pslt.tile([P, 2 * N512], F32, tag="lt")
            nc.tensor.matmul(
                bt[:, 0:N512], wp_sb[:, KT - 1, m * P:(m + 1) * P],
                xT[:, KT - 1, :], start=True, stop=True,
            )
            pt = ppA.pop(m)
            nc.vector.tensor_scalar(
                out=xpT[:, m, :], in0=pt[:, :],
                scalar1=bp_sb[:, m:m + 1], scalar2=None, op0=add,
            )
            nc.vector.tensor_add(xpT[:, m, :], bt[:, 0:N512], xpT[:, m, :])

        for h in range(H):
            emit_pv(h)
            if h + 4 < H:
                emit_qk(h + 4)
            if h < ktv:
                emit_v_chunk(h, 1)
            if h == H - 3:
                emit_pproj_A(0)
            elif h == H - 2:
                emit_pproj_A(1)

        # ---- output projection + MLP ----
        emit_pproj_B(0)
        emit_pproj_B(1)
        for m in range(2, KT):
            pt = ps.tile([P, N512], F32, tag="mm")
            for k in range(KT):
                nc.tensor.matmul(
                    pt[:, :], wp_sb[:, k, m * P:(m + 1) * P], xT[:, k, :],
                    start=(k == 0), stop=(k == KT - 1),
                )
            nc.vector.tensor_scalar(
                out=xpT[:, m, :], in0=pt[:, :],
                scalar1=bp_sb[:, m:m + 1], scalar2=None, op0=add,
            )

        for m in range(KT):
            pt = ps.tile([P, N512], F32, tag="mm")
            for k in range(KT):
                nc.tensor.matmul(
                    pt[:, :], w1_sb[:, k, m * P:(m + 1) * P], xpT[:, k, :],
                    start=(k == 0), stop=(k == KT - 1),
                )
            nc.scalar.activation(
                out=h1T[:, m, :], in_=pt[:, :], func=Act.Gelu,
                bias=b1_sb[:, m:m + 1], scale=1.0,
            )

        for m in range(KT):
            pt = ps.tile([P, N512], F32, tag="mm")
            for k in range(KT):
                nc.tensor.matmul(
                    pt[:, :], w2_sb[:, k, m * P:(m + 1) * P], h1T[:, k, :],
                    start=(k == 0), stop=(k == KT - 1),
                )
            ot = outp.tile([P, N512], F32, tag="o")
            nc.vector.scalar_tensor_tensor(
                out=ot[:, :], in0=pt[:, :], scalar=b2_sb[:, m:m + 1],
                in1=xpT[:, m, :], op0=add, op1=add,
            )
            nc.sync.dma_start(out[m * P:(m + 1) * P, :], ot[:, :])

    nc.compile()
    return nc


_prog_cache = {}


def _get_program(vp):
    if vp not in _prog_cache:
        _prog_cache[vp] = build_program(vp)
    return _prog_cache[vp]


def _repack_w_cols(W):
    # [C, C] -> [m, p, k, j] with [m, p, k, j] = W[k*128+p, m*128+j]
    return np.ascontiguousarray(
        W.reshape(KT, P, KT, P).transpose(2, 1, 0, 3))


def make_in_maps(inputs, vp):
    q = np.asarray(inputs["query"], np.float32)
    k = np.asarray(inputs["key"], np.float32)
    v = np.asarray(inputs["value"], np.float32)
    mask = np.asarray(inputs["mask"])
    Wq = np.asarray(inputs["Wq"], np.float32) * SCALE
    bq = np.asarray(inputs["bq"], np.float32) * SCALE
    Wk = np.asarray(inputs["Wk"], np.float32)
    Wv = np.asarray(inputs["Wv"], np.float32)
    bv = np.asarray(inputs["bv"], np.float32)
    Wp = np.asarray(inputs["Wp"], np.float32)
    bp = np.asarray(inputs["bp"], np.float32)
    W1 = np.asarray(inputs["W1"], np.float32)
    b1 = np.asarray(inputs["b1"], np.float32)
    W2 = np.asarray(inputs["W2"], np.float32)
    b2 = np.asarray(inputs["b2"], np.float32)

    bp_eff = bp + bv @ Wp

    shared = {
        "wq": _repack_w_cols(Wq.astype(NPBF16)),
        "wk": _repack_w_cols(Wk.astype(NPBF16)),
        "wv": np.ascontiguousarray(Wv.astype(NPBF16)),
        "wp": np.ascontiguousarray(Wp.astype(NPBF16)),
        "w1": np.ascontiguousarray(W1.astype(NPBF16)),
        "w2": np.ascontiguousarray(W2.astype(NPBF16)),
    }

    def pack_cols(vec):      # [C] -> [P, KT] with [p, j] = vec[j*128+p]
        return np.asarray(vec, np.float32).reshape(KT, P).T

    base = np.zeros((P, 4, KT), np.float32)
    for i, vec in enumerate((bp_eff, b1, b2, bq)):
        base[:, i, :] = pack_cols(vec)

    combined = (mask[:, :S] != 0) | (mask[:, S:2 * S] != 0)   # [B, S]
    in_maps = []
    kt_b, vt_b, mrow_b = {}, {}, {}
    for b in range(B):
        idx = np.nonzero(combined[b])[0]
        nv = len(idx)
        kp = np.zeros((C, vp), NPBF16)
        vpk = np.zeros((C, vp), NPBF16)
        kp[:, :nv] = k[b][idx, :].T.astype(NPBF16)
        vpk[:, :nv] = v[b][idx, :].T.astype(NPBF16)
        kt_b[b] = np.ascontiguousarray(kp)
        vt_b[b] = np.ascontiguousarray(vpk)
        mrow = np.full(vp, MASK_NEG, NPBF16)
        mrow[:nv] = 0.0
        mrow_b[b] = mrow

    for core in range(NCORES):
        b, qs = divmod(core, 2)
        m = dict(shared)
        m["qt_in"] = np.ascontiguousarray(
            q[b, qs * SQ:(qs + 1) * SQ, :].T.astype(NPBF16))
        m["kt_in"] = kt_b[b]
        m["vt_in"] = vt_b[b]
        m["maskrow"] = mrow_b[b]
        m["bvecs"] = np.ascontiguousarray(base)
        in_maps.append(m)
    return in_maps


def run(inputs, trace=False, trace_cores=None):
    mask = np.asarray(inputs["mask"])
    combined = (mask[:, :S] != 0) | (mask[:, S:2 * S] != 0)
    maxv = int(combined.sum(1).max())
    vp = max(P, -(-maxv // P) * P)       # round up to a multiple of 128
    nc = _get_program(vp)
    in_maps = make_in_maps(inputs, vp)
    res = run_bass_kernel_spmd(
        nc, in_maps, core_ids=list(range(NCORES)),
        trace=trace, trace_cores=trace_cores,
    )
    outfull = np.empty((B, S, C), np.float32)
    for core in range(NCORES):
        b, qs = divmod(core, 2)
        outfull[b, qs * SQ:(qs + 1) * SQ, :] = res.results[core]["out"].T
    return outfull, res


def kernel(**inputs):
    outfull, _ = run(inputs)
    return outfull


# revision 17
# speedup vs baseline: 1.0050x; 1.0050x over previous
"""Trainium2 Bass kernel for nn_CrossAttention (B=4, S=1024, C=1024, H=16).

Sharding: 8 cores = (batch b in 0..4) x (query-half qs in 0..2). Each core
computes, for its 512 query rows of batch b: the Q projection, K/V
projections over the *packed valid* key positions, masked-softmax attention
over all 16 heads, the output projection, and the MLP with residual. No
collectives.

Sparsity: the mask is per (batch, key-position) and zeroes out ~25% of key
columns entirely, so the host packs K/V to the valid positions only
(padded to a multiple of 128, VP columns). Padding columns get maskrow
-30000 -> exp==0, and zero V -> no contribution.

Bias folding: softmax over kpos makes any per-query constant cancel, so
the K-side bias bk never matters; logits == (q0+bq).k0 + mask (mod
constants). bq is added during the Q-projection psum eviction
(tensor_scalar), bv is folded into bp_eff = bp + bv@Wp on the host.

DMA layout: wq/wk are host-repacked into per-m column blocks
[m][p][k][j] so each block is one dense 2KB-per-partition transfer, and
loads are spread over four engine queues (sync/scalar/vector/gpsimd)
with the tiny constants first so nothing queues behind megabyte loads.

All activations flow transposed (contraction dim on SBUF partitions):
  qT = Wq'^T @ queryT + bq'      [c_out, q]   (Wq' = Wq*scale)
  kT = Wk^T  @ keyT_packed       [c_out, vp]
  v  = valueT_packed^T @ Wv      [vp, c_out]
  per head h with augmented rows: qTaug = [qT_h ; 1], kTaug = [kT_h ; mask]
  LTh = kTaug^T @ qTaug          [vp, q] logits^T;  PTh = exp(LTh)
  oTh = [v_h | 1]^T @ PTh        [65, q]; row 64 = softmax denominator
  xT[h] = oTh[0:64] * bcast(1/denom)
  xpT = Wp^T @ xT + bp_eff;  h1T = gelu(W1^T @ xpT + b1)
  outT = xpT + W2^T @ h1T + b2
"""

from contextlib import ExitStack

import numpy as np

import concourse.bass as bass
import concourse.tile as tile
from concourse import bacc, mybir
from concourse.bass_utils import run_bass_kernel_spmd

B, S, C, H = 4, 1024, 1024, 16
HD = C // H          # 64
SCALE = HD ** -0.5
P = 128              # SBUF partitions
SQ = S // 2          # 512 query rows per core
NCORES = 8
KT = C // P          # 8 contraction tiles of 128
N512 = 512
MASK_NEG = -30000.0

F32 = mybir.dt.float32
BF16 = mybir.dt.bfloat16
NPBF16 = mybir.dt.np(BF16)


def build_program(vp):
    ktv = vp // P    # packed key-position tiles
    nc = bacc.Bacc(None, target_bir_lowering=False, debug=False)

    # wq/wk repacked on host: [m, p, k, j] = W[k*128+p, m*128+j]
    wq = nc.dram_tensor("wq", [KT, P, KT, P], BF16, kind="ExternalInput")
    wk = nc.dram_tensor("wk", [KT, P, KT, P], BF16, kind="ExternalInput")
    wv = nc.dram_tensor("wv", [C, C], BF16, kind="ExternalInput")
    wp = nc.dram_tensor("wp", [C, C], BF16, kind="ExternalInput")
    w1 = nc.dram_tensor("w1", [C, C], BF16, kind="ExternalInput")
    w2 = nc.dram_tensor("w2", [C, C], BF16, kind="ExternalInput")
    qt_in = nc.dram_tensor("qt_in", [C, SQ], BF16, kind="ExternalInput")
    kt_in = nc.dram_tensor("kt_in", [C, vp], BF16, kind="ExternalInput")
    vt_in = nc.dram_tensor("vt_in", [C, vp], BF16, kind="ExternalInput")
    # per-channel vectors packed to [P, 4, KT]: i=0..3 -> bp_eff, b1, b2, bq'
    bvecs = nc.dram_tensor("bvecs", [P, 4, KT], F32, kind="ExternalInput")
    maskrow = nc.dram_tensor("maskrow", [vp], BF16, kind="ExternalInput")
    out = nc.dram_tensor("out", [C, SQ], F32, kind="ExternalOutput")

    add = mybir.AluOpType.add
    Act = mybir.ActivationFunctionType

    with tile.TileContext(nc) as tc, ExitStack() as ctx:
        const = ctx.enter_context(tc.tile_pool(name="const", bufs=1))
        wcol = ctx.enter_context(tc.tile_pool(name="wcol", bufs=1))
        wfull = ctx.enter_context(tc.tile_pool(name="wfull", bufs=3))
        acts = ctx.enter_context(tc.tile_pool(name="acts", bufs=1))
        ptp = ctx.enter_context(tc.tile_pool(name="ptp", bufs=4))
        smal = ctx.enter_context(tc.tile_pool(name="smal", bufs=2))
        outp = ctx.enter_context(tc.tile_pool(name="outp", bufs=3))
        ps = ctx.enter_context(tc.tile_pool(name="ps", bufs=2, space="PSUM"))
        pslt = ctx.enter_context(tc.tile_pool(name="pslt", bufs=2, space="PSUM"))
        pvp = ctx.enter_context(tc.tile_pool(name="pvp", bufs=2, space="PSUM"))

        # ---- tiles ----
        qin = acts.tile([P, KT, SQ], BF16, tag="qin_xT")
        kin = acts.tile([P, KT, vp], BF16, tag="kin_h1T")
        vin = acts.tile([P, KT, vp], BF16, tag="vin_xpT")
        bv_sb = const.tile([P, 4, KT], F32, tag="bvecs")
        bp_sb = bv_sb[:, 0, :]
        b1_sb = bv_sb[:, 1, :]
        b2_sb = bv_sb[:, 2, :]
        bq_sb = bv_sb[:, 3, :]
        qTa = acts.tile([HD + 1, H, SQ], BF16, tag="qTa")
        kTa = acts.tile([HD + 1, H, vp], BF16, tag="kTa")
        vaug = acts.tile([P, ktv, H * 65], BF16, tag="va")  # [kpos, h*(64|1)]
        xT = acts.tile([P, KT, SQ], BF16, tag="qin_xT")        # [c, q] attn out
        xpT = acts.tile([P, KT, SQ], BF16, tag="vin_xpT")      # [c', q] proj out
        h1T = acts.tile([P, KT, SQ], BF16, tag="kin_h1T")      # [c_h, q] hidden

        # ---- DMA queues: constants first on vector; inputs split over
        # sync+scalar; wq on gpsimd; wk on sync after the inputs ----
        nc.scalar.dma_start(bv_sb[:, :, :], bvecs[:, :, :])
        mr = maskrow.ap()
        mrow_bcast = bass.AP(tensor=mr.tensor, offset=mr.offset,
                             ap=[[0, H]] + list(mr.ap))
        nc.scalar.dma_start(kTa[HD:HD + 1, :, :], mrow_bcast)

        qin_sr = qt_in.rearrange("(k p) n -> p k n", p=P)
        nc.sync.dma_start(qin[:, 0:4, :], qin_sr[:, 0:4, :])
        nc.scalar.dma_start(qin[:, 4:8, :], qin_sr[:, 4:8, :])

        def load_w_cols(w_dram, eng, eng2=None, split=KT):
            tiles = []
            for m in range(KT):
                wm = wcol.tile([P, KT, P], BF16, tag=f"wc{m}")
                e = eng if (m < split or eng2 is None) else eng2
                e.dma_start(wm[:, :, :], w_dram[m, :, :, :])
                tiles.append(wm)
            return tiles

        def load_w_full(w_dram, eng):
            wsb = wfull.tile([P, KT, C], BF16, tag="w")
            eng.dma_start(wsb[:, :, :], w_dram.rearrange("(k p) n -> p k n", p=P))
            return wsb

        wqm = load_w_cols(wq, nc.gpsimd, nc.scalar, split=6)

        kin_sr = kt_in.rearrange("(k p) n -> p k n", p=P)
        nc.sync.dma_start(kin[:, 0:4, :], kin_sr[:, 0:4, :])
        nc.scalar.dma_start(kin[:, 4:8, :], kin_sr[:, 4:8, :])
        vin_sr = vt_in.rearrange("(k p) n -> p k n", p=P)
        nc.sync.dma_start(vin[:, 0:4, :], vin_sr[:, 0:4, :])
        nc.scalar.dma_start(vin[:, 4:8, :], vin_sr[:, 4:8, :])

        # wfull rotation (bufs=3): wv, wp, w1, then w2 reuses wv's buffer
        # once the V projection has consumed it (~halfway in); its sync
        # issue therefore blocks only the late output DMAs.
        wv_sb = load_w_full(wv, nc.gpsimd)
        wp_sb = load_w_full(wp, nc.gpsimd)
        w1_sb = load_w_full(w1, nc.gpsimd)

        nc.vector.memset(qTa[HD:HD + 1, :, :], 1.0)        # ones row (64)
        vaug_h = vaug.rearrange("p k (h e) -> p k h e", e=65)
        for kt in range(ktv):
            nc.vector.memset(vaug_h[:, kt, :, 64:65], 1.0)

        # ---- PE warmup: the tensor engine p-state ramps to full clock only
        # after ~3us of continuous execution; burn the initial qin DMA wait
        # on throwaway matmuls so the Q projection runs at full speed ----
        warm = const.tile([P, N512], BF16, tag="warm")
        nc.vector.memset(warm[:, :], 0.0)
        wt = ps.tile([P, N512], F32, tag="mm")
        for _ in range(12):
            nc.tensor.matmul(wt[:, :], warm[:, 0:P], warm[:, :],
                             start=True, stop=True)

        # ---- Q projection (transposed output, + bq during eviction) ----
        for m in range(KT):
            pt = ps.tile([P, N512], F32, tag="mm")
            for k in range(KT):
                nc.tensor.matmul(
                    pt[:, :], wqm[m][:, k, :], qin[:, k, :],
                    start=(k == 0), stop=(k == KT - 1),
                )
            nc.vector.tensor_scalar(
                out=qTa[0:HD, 2 * m, :], in0=pt[0:HD, :],
                scalar1=bq_sb[0:HD, m:m + 1], scalar2=None, op0=add,
            )
            nc.vector.tensor_scalar(
                out=qTa[0:HD, 2 * m + 1, :], in0=pt[HD:P, :],
                scalar1=bq_sb[HD:P, m:m + 1], scalar2=None, op0=add,
            )

        wkm = load_w_cols(wk, nc.sync)
        w2_sb = load_w_full(w2, nc.sync)

        # n-chunks of the packed key axis (512 then the remainder)
        nchunks = []
        n0 = 0
        while n0 < vp:
            nsz = min(N512, vp - n0)
            nchunks.append((n0, nsz))
            n0 += nsz

        pTts = {}

        def emit_qk(h):
            pTt = ptp.tile([P, ktv, N512], BF16, tag="pt")
            pTts[h] = pTt
            for t in range((ktv + 1) // 2):
                lt = pslt.tile([P, 2 * N512], F32, tag="lt")
                kts = [kt for kt in (2 * t, 2 * t + 1) if kt < ktv]
                for kt in kts:
                    nc.tensor.matmul(
                        lt[:, (kt % 2) * N512:((kt % 2) + 1) * N512],
                        kTa[0:HD + 1, h, kt * P:(kt + 1) * P],
                        qTa[0:HD + 1, h, :],
                        start=True, stop=True,
                    )
                nc.scalar.activation(
                    out=pTt[:, 2 * t:2 * t + len(kts), :],
                    in_=lt[:, 0:len(kts) * N512].rearrange(
                        "p (t n) -> p t n", n=N512),
                    func=Act.Exp,
                )

        # ---- K projection; QK for heads 0-2 interleaved so the scalar
        # engine's exp stream starts early ----
        for m in range(KT):
            for n0, nsz in nchunks:
                pt = ps.tile([P, N512], F32, tag="mm")
                for k in range(KT):
                    nc.tensor.matmul(
                        pt[:, 0:nsz],
                        wkm[m][:, k, :],
                        kin[:, k, n0:n0 + nsz],
                        start=(k == 0), stop=(k == KT - 1),
                    )
                ns = slice(n0, n0 + nsz)
                nc.vector.tensor_copy(kTa[0:HD, 2 * m, ns], pt[0:HD, 0:nsz])
                nc.vector.tensor_copy(kTa[0:HD, 2 * m + 1, ns], pt[HD:P, 0:nsz])
            if m < 3:
                emit_qk(m)

        # ---- attention: V projection + remaining QK + PV pipeline ----
        def emit_v_chunk(m, n):
            pt = ps.tile([P, N512], F32, tag="mm")
            for k in range(KT):
                nc.tensor.matmul(
                    pt[:, :],
                    vin[:, k, m * P:(m + 1) * P],
                    wv_sb[:, k, n * N512:(n + 1) * N512],
                    start=(k == 0), stop=(k == KT - 1),
                )
            nc.vector.tensor_copy(
                vaug_h[:, m, 8 * n:8 * n + 8, 0:64],
                pt[:, :].rearrange("p (h d) -> p h d", d=HD),
            )

        def emit_pv(h):
            hp = (h % 2) * HD
            hm = h // 2
            pTt = pTts.pop(h)
            pv = pvp.tile([HD + 1, N512], F32, tag="pv")
            for kt in range(ktv):
                nc.tensor.matmul(
                    pv[:, :],
                    vaug[:, kt, h * 65:(h + 1) * 65],
                    pTt[:, kt, :],
                    start=(kt == 0), stop=(kt == ktv - 1),
                )
            rc = smal.tile([1, N512], F32, tag="rc")
            bc = smal.tile([HD, N512], F32, tag="bc")
            # denom >= exp(min valid logit) > 0 (each batch has >=1 valid
            # key position), so the fast reciprocal's denorm/zero edge
            # cases cannot occur
            nc.vector.tensor_copy(rc[0:1, :], pv[HD:HD + 1, :])
            nc.vector.reciprocal_approx_fast(out=rc[0:1, :], in_=rc[0:1, :])
            nc.gpsimd.partition_broadcast(bc[:, :], rc[0:1, :])
            nc.vector.tensor_mul(xT[hp:hp + HD, hm, :], pv[0:HD, :], bc[:, :])

        # V chunks n=0 first (PV_0..7 read the n=0 half of vaug), then the
        # PV pipeline with the remaining QKs and V chunks interleaved.
        # pTt window stays <= 4 tiles. The last PV slots are filled with
        # the head-independent part of the first two P-proj chains.
        for m in range(ktv):
            emit_v_chunk(m, 0)
            if m == 0:
                emit_qk(3)

        ppA = {}

        def emit_pproj_A(m):
            # k=0..6 of P-proj chain m: independent of the last PV evicts
            pt = ps.tile([P, N512], F32, tag="mm")
            ppA[m] = pt
            for k in range(KT - 1):
                nc.tensor.matmul(
                    pt[:, :], wp_sb[:, k, m * P:(m + 1) * P], xT[:, k, :],
                    start=(k == 0), stop=(k == KT - 2),
                )

        def emit_pproj_B(m):
            # k=7 into a spare pslt half; A evicts (+bias) then B adds in
            # (a vector op may read only one PSUM operand)
            bt = pslt.tile([P, 2 * N512], F32, tag="lt")
            nc.tensor.matmul(
                bt[:, 0:N512], wp_sb[:, KT - 1, m * P:(m + 1) * P],
                xT[:, KT - 1, :], start=True, stop=True,
            )
            pt = ppA.pop(m)
            nc.vector.tensor_scalar(
                out=xpT[:, m, :], in0=pt[:, :],
                scalar1=bp_sb[:, m:m + 1], scalar2=None, op0=add,
            )
            nc.vector.tensor_add(xpT[:, m, :], bt[:, 0:N512], xpT[:, m, :])

        for h in range(H):
            emit_pv(h)
            if h + 4 < H:
                emit_qk(h + 4)
            if h < ktv:
                emit_v_chunk(h, 1)
            if h == H - 3:
                emit_pproj_A(0)
            elif h == H - 2:
                emit_pproj_A(1)

        # ---- output projection + MLP ----
        emit_pproj_B(0)
        emit_pproj_B(1)
        for m in range(2, KT):
            pt = ps.tile([P, N512], F32, tag="mm")
            for k in range(KT):
                nc.tensor.matmul(
                    pt[:, :], wp_sb[:, k, m * P:(m + 1) * P], xT[:, k, :],
                    start=(k == 0), stop=(k == KT - 1),
                )
            nc.vector.tensor_scalar(
                out=xpT[:, m, :], in0=pt[:, :],
                scalar1=bp_sb[:, m:m + 1], scalar2=None, op0=add,
            )

        for m in range(KT):
            pt = ps.tile([P, N512], F32, tag="mm")
            for k in range(KT):
                nc.tensor.matmul(
                    pt[:, :], w1_sb[:, k, m * P:(m + 1) * P], xpT[:, k, :],
                    start=(k == 0), stop=(k == KT - 1),
                )
            nc.scalar.activation(
                out=h1T[:, m, :], in_=pt[:, :], func=Act.Gelu,
                bias=b1_sb[:, m:m + 1], scale=1.0,
            )

        for m in range(KT):
            pt = ps.tile([P, N512], F32, tag="mm")
            for k in range(KT):
                nc.tensor.matmul(
                    pt[:, :], w2_sb[:, k, m * P:(m + 1) * P], h1T[:, k, :],
                    start=(k == 0), stop=(k == KT - 1),
                )
            ot = outp.tile([P, N512], F32, tag="o")
            nc.vector.scalar_tensor_tensor(
                out=ot[:, :], in0=pt[:, :], scalar=b2_sb[:, m:m + 1],
                in1=xpT[:, m, :], op0=add, op1=add,
            )
            nc.sync.dma_start(out[m * P:(m + 1) * P, :], ot[:, :])

    nc.compile()
    return nc


_prog_cache = {}


def _get_program(vp):
    if vp not in _prog_cache:
        _prog_cache[vp] = build_program(vp)
    return _prog_cache[vp]


def _repack_w_cols(W):
    # [C, C] -> [m, p, k, j] with [m, p, k, j] = W[k*128+p, m*128+j]
    return np.ascontiguousarray(
        W.reshape(KT, P, KT, P).transpose(2, 1, 0, 3))


def make_in_maps(inputs, vp):
    q = np.asarray(inputs["query"], np.float32)
    k = np.asarray(inputs["key"], np.float32)
    v = np.asarray(inputs["value"], np.float32)
    mask = np.asarray(inputs["mask"])
    Wq = np.asarray(inputs["Wq"], np.float32) * SCALE
    bq = np.asarray(inputs["bq"], np.float32) * SCALE
    Wk = np.asarray(inputs["Wk"], np.float32)
    Wv = np.asarray(inputs["Wv"], np.float32)
    bv = np.asarray(inputs["bv"], np.float32)
    Wp = np.asarray(inputs["Wp"], np.float32)
    bp = np.asarray(inputs["bp"], np.float32)
    W1 = np.asarray(inputs["W1"], np.float32)
    b1 = np.asarray(inputs["b1"], np.float32)
    W2 = np.asarray(inputs["W2"], np.float32)
    b2 = np.asarray(inputs["b2"], np.float32)

    bp_eff = bp + bv @ Wp

    shared = {
        "wq": _repack_w_cols(Wq.astype(NPBF16)),
        "wk": _repack_w_cols(Wk.astype(NPBF16)),
        "wv": np.ascontiguousarray(Wv.astype(NPBF16)),
        "wp": np.ascontiguousarray(Wp.astype(NPBF16)),
        "w1": np.ascontiguousarray(W1.astype(NPBF16)),
        "w2": np.ascontiguousarray(W2.astype(NPBF16)),
    }

    def pack_cols(vec):      # [C] -> [P, KT] with [p, j] = vec[j*128+p]
        return np.asarray(vec, np.float32).reshape(KT, P).T

    base = np.zeros((P, 4, KT), np.float32)
    for i, vec in enumerate((bp_eff, b1, b2, bq)):
        base[:, i, :] = pack_cols(vec)

    combined = (mask[:, :S] != 0) | (mask[:, S:2 * S] != 0)   # [B, S]
    in_maps = []
    kt_b, vt_b, mrow_b = {}, {}, {}
    for b in range(B):
        idx = np.nonzero(combined[b])[0]
        nv = len(idx)
        kp = np.zeros((C, vp), NPBF16)
        vpk = np.zeros((C, vp), NPBF16)
        kp[:, :nv] = k[b][idx, :].T.astype(NPBF16)
        vpk[:, :nv] = v[b][idx, :].T.astype(NPBF16)
        kt_b[b] = np.ascontiguousarray(kp)
        vt_b[b] = np.ascontiguousarray(vpk)
        mrow = np.full(vp, MASK_NEG, NPBF16)
        mrow[:nv] = 0.0
        mrow_b[b] = mrow

    for core in range(NCORES):
        b, qs = divmod(core, 2)
        m = dict(shared)
        m["qt_in"] = np.ascontiguousarray(
            q[b, qs * SQ:(qs + 1) * SQ, :].T.astype(NPBF16))
        m["kt_in"] = kt_b[b]
        m["vt_in"] = vt_b[b]
        m["maskrow"] = mrow_b[b]
        m["bvecs"] = np.ascontiguousarray(base)
        in_maps.append(m)
    return in_maps


def run(inputs, trace=False, trace_cores=None):
    mask = np.asarray(inputs["mask"])
    combined = (mask[:, :S] != 0) | (mask[:, S:2 * S] != 0)
    maxv = int(combined.sum(1).max())
    vp = max(P, -(-maxv // P) * P)       # round up to a multiple of 128
    nc = _get_program(vp)
    in_maps = make_in_maps(inputs, vp)
    res = run_bass_kernel_spmd(
        nc, in_maps, core_ids=list(range(NCORES)),
        trace=trace, trace_cores=trace_cores,
    )
    outfull = np.empty((B, S, C), np.float32)
    for core in range(NCORES):
        b, qs = divmod(core, 2)
        outfull[b, qs * SQ:(qs + 1) * SQ, :] = res.results[core]["out"].T
    return outfull, res


def kernel(**inputs):
    outfull, _ = run(inputs)
    return outfull


# revision 19
# speedup vs baseline: 1.0181x; 1.0131x over previous
"""Trainium2 Bass kernel for nn_CrossAttention (B=4, S=1024, C=1024, H=16).

Sharding: 8 cores = (batch b in 0..4) x (query-half qs in 0..2). Each core
computes, for its 512 query rows of batch b: the Q projection, K/V
projections over the *packed valid* key positions, masked-softmax attention
over all 16 heads, the output projection, and the MLP with residual. No
collectives.

Sparsity: the mask is per (batch, key-position) and zeroes out ~25% of key
columns entirely, so the host packs K/V to the valid positions only
(padded to a multiple of 128, VP columns). Padding columns get maskrow
-30000 -> exp==0, and zero V -> no contribution.

Bias folding: softmax over kpos makes any per-query constant cancel, so
the K-side bias bk never matters; logits == (q0+bq).k0 + mask (mod
constants). bq is added during the Q-projection psum eviction
(tensor_scalar), bv is folded into bp_eff = bp + bv@Wp on the host.

DMA layout: wq/wk are host-repacked into per-m column blocks
[m][p][k][j] so each block is one dense 2KB-per-partition transfer, and
loads are spread over four engine queues (sync/scalar/vector/gpsimd)
with the tiny constants first so nothing queues behind megabyte loads.

All activations flow transposed (contraction dim on SBUF partitions):
  qT = Wq'^T @ queryT + bq'      [c_out, q]   (Wq' = Wq*scale)
  kT = Wk^T  @ keyT_packed       [c_out, vp]
  v  = valueT_packed^T @ Wv      [vp, c_out]
  per head h with augmented rows: qTaug = [qT_h ; 1], kTaug = [kT_h ; mask]
  LTh = kTaug^T @ qTaug          [vp, q] logits^T;  PTh = exp(LTh)
  oTh = [v_h | 1]^T @ PTh        [65, q]; row 64 = softmax denominator
  xT[h] = oTh[0:64] * bcast(1/denom)
  xpT = Wp^T @ xT + bp_eff;  h1T = gelu(W1^T @ xpT + b1)
  outT = xpT + W2^T @ h1T + b2
"""

from contextlib import ExitStack

import numpy as np

import concourse.bass as bass
import concourse.tile as tile
from concourse import bacc, mybir
from concourse.bass_utils import run_bass_kernel_spmd

B, S, C, H = 4, 1024, 1024, 16
HD = C // H          # 64
SCALE = HD ** -0.5
P = 128              # SBUF partitions
SQ = S // 2          # 512 query rows per core
NCORES = 8
KT = C // P          # 8 contraction tiles of 128
N512 = 512
MASK_NEG = -30000.0

F32 = mybir.dt.float32
BF16 = mybir.dt.bfloat16
NPBF16 = mybir.dt.np(BF16)


def build_program(vp):
    ktv = vp // P    # packed key-position tiles
    nc = bacc.Bacc(None, target_bir_lowering=False, debug=False)

    # wq/wk repacked on host: [m, p, k, j] = W[k*128+p, m*128+j]
    wq = nc.dram_tensor("wq", [KT, P, KT, P], BF16, kind="ExternalInput")
    wk = nc.dram_tensor("wk", [KT, P, KT, P], BF16, kind="ExternalInput")
    wv = nc.dram_tensor("wv", [C, C], BF16, kind="ExternalInput")
    wp = nc.dram_tensor("wp", [C, C], BF16, kind="ExternalInput")
    w1 = nc.dram_tensor("w1", [C, C], BF16, kind="ExternalInput")
    w2 = nc.dram_tensor("w2", [C, C], BF16, kind="ExternalInput")
    qt_in = nc.dram_tensor("qt_in", [C, SQ], BF16, kind="ExternalInput")
    kt_in = nc.dram_tensor("kt_in", [C, vp], BF16, kind="ExternalInput")
    vt_in = nc.dram_tensor("vt_in", [C, vp], BF16, kind="ExternalInput")
    # per-channel vectors packed to [P, 4, KT]: i=0..3 -> bp_eff, b1, b2, bq'
    bvecs = nc.dram_tensor("bvecs", [P, 4, KT], F32, kind="ExternalInput")
    maskrow = nc.dram_tensor("maskrow", [vp], BF16, kind="ExternalInput")
    out = nc.dram_tensor("out", [C, SQ], F32, kind="ExternalOutput")

    add = mybir.AluOpType.add
    Act = mybir.ActivationFunctionType

    with tile.TileContext(nc) as tc, ExitStack() as ctx:
        const = ctx.enter_context(tc.tile_pool(name="const", bufs=1))
        wcol = ctx.enter_context(tc.tile_pool(name="wcol", bufs=1))
        wfull = ctx.enter_context(tc.tile_pool(name="wfull", bufs=3))
        acts = ctx.enter_context(tc.tile_pool(name="acts", bufs=1))
        ptp = ctx.enter_context(tc.tile_pool(name="ptp", bufs=4))
        smal = ctx.enter_context(tc.tile_pool(name="smal", bufs=2))
        outp = ctx.enter_context(tc.tile_pool(name="outp", bufs=3))
        ps = ctx.enter_context(tc.tile_pool(name="ps", bufs=2, space="PSUM"))
        pslt = ctx.enter_context(tc.tile_pool(name="pslt", bufs=2, space="PSUM"))
        pvp = ctx.enter_context(tc.tile_pool(name="pvp", bufs=2, space="PSUM"))

        # ---- tiles ----
        qin = acts.tile([P, KT, SQ], BF16, tag="qin_xT")
        kin = acts.tile([P, KT, vp], BF16, tag="kin_h1T")
        vin = acts.tile([P, KT, vp], BF16, tag="vin_xpT")
        bv_sb = const.tile([P, 4, KT], F32, tag="bvecs")
        bp_sb = bv_sb[:, 0, :]
        b1_sb = bv_sb[:, 1, :]
        b2_sb = bv_sb[:, 2, :]
        bq_sb = bv_sb[:, 3, :]
        qTa = acts.tile([HD + 1, H, SQ], BF16, tag="qTa")
        kTa = acts.tile([HD + 1, H, vp], BF16, tag="kTa")
        vaug = acts.tile([P, ktv, H * 65], BF16, tag="va")  # [kpos, h*(64|1)]
        xT = acts.tile([P, KT, SQ], BF16, tag="qin_xT")        # [c, q] attn out
        xpT = acts.tile([P, KT, SQ], BF16, tag="vin_xpT")      # [c', q] proj out
        h1T = acts.tile([P, KT, SQ], BF16, tag="kin_h1T")      # [c_h, q] hidden

        # ---- DMA queues: constants first on vector; inputs split over
        # sync+scalar; wq on gpsimd; wk on sync after the inputs ----
        nc.scalar.dma_start(bv_sb[:, :, :], bvecs[:, :, :])
        mr = maskrow.ap()
        mrow_bcast = bass.AP(tensor=mr.tensor, offset=mr.offset,
                             ap=[[0, H]] + list(mr.ap))
        nc.scalar.dma_start(kTa[HD:HD + 1, :, :], mrow_bcast)

        qin_sr = qt_in.rearrange("(k p) n -> p k n", p=P)
        nc.sync.dma_start(qin[:, 0:4, :], qin_sr[:, 0:4, :])
        nc.scalar.dma_start(qin[:, 4:8, :], qin_sr[:, 4:8, :])

        def load_w_cols(w_dram, eng, eng2=None, split=KT):
            tiles = []
            for m in range(KT):
                wm = wcol.tile([P, KT, P], BF16, tag=f"wc{m}")
                e = eng if (m < split or eng2 is None) else eng2
                e.dma_start(wm[:, :, :], w_dram[m, :, :, :])
                tiles.append(wm)
            return tiles

        def load_w_full(w_dram, eng):
            wsb = wfull.tile([P, KT, C], BF16, tag="w")
            eng.dma_start(wsb[:, :, :], w_dram.rearrange("(k p) n -> p k n", p=P))
            return wsb

        wqm = load_w_cols(wq, nc.gpsimd)

        kin_sr = kt_in.rearrange("(k p) n -> p k n", p=P)
        nc.sync.dma_start(kin[:, 0:4, :], kin_sr[:, 0:4, :])
        nc.scalar.dma_start(kin[:, 4:8, :], kin_sr[:, 4:8, :])
        vin_sr = vt_in.rearrange("(k p) n -> p k n", p=P)
        nc.sync.dma_start(vin[:, 0:4, :], vin_sr[:, 0:4, :])
        nc.scalar.dma_start(vin[:, 4:8, :], vin_sr[:, 4:8, :])

        # wfull rotation (bufs=3): wv, wp, w1, then w2 reuses wv's buffer
        # once the V projection has consumed it (~halfway in); its sync
        # issue therefore blocks only the late output DMAs.
        wv_sb = load_w_full(wv, nc.gpsimd)
        wp_sb = load_w_full(wp, nc.gpsimd)
        w1_sb = load_w_full(w1, nc.gpsimd)

        # ---- PE warmup: the tensor engine p-state ramps to full clock only
        # after ~3us of continuous execution; burn the initial qin DMA wait
        # on throwaway matmuls so the Q projection runs at full speed.
        # (warm's memset goes first on the vector queue — the vaug memset
        # takes ~7us and would otherwise gate the warmup.) ----
        warm = const.tile([P, N512], BF16, tag="warm")
        nc.vector.memset(warm[:, :], 0.0)
        wt = ps.tile([P, N512], F32, tag="mm")
        for _ in range(12):
            nc.tensor.matmul(wt[:, :], warm[:, 0:P], warm[:, :],
                             start=True, stop=True)

        nc.vector.memset(qTa[HD:HD + 1, :, :], 1.0)        # ones row (64)
        vaug_h = vaug.rearrange("p k (h e) -> p k h e", e=65)
        for kt in range(ktv):
            nc.vector.memset(vaug_h[:, kt, :, 64:65], 1.0)

        # ---- Q projection (transposed output, + bq during eviction) ----
        for m in range(KT):
            pt = ps.tile([P, N512], F32, tag="mm")
            for k in range(KT):
                nc.tensor.matmul(
                    pt[:, :], wqm[m][:, k, :], qin[:, k, :],
                    start=(k == 0), stop=(k == KT - 1),
                )
            nc.vector.tensor_scalar(
                out=qTa[0:HD, 2 * m, :], in0=pt[0:HD, :],
                scalar1=bq_sb[0:HD, m:m + 1], scalar2=None, op0=add,
            )
            nc.vector.tensor_scalar(
                out=qTa[0:HD, 2 * m + 1, :], in0=pt[HD:P, :],
                scalar1=bq_sb[HD:P, m:m + 1], scalar2=None, op0=add,
            )

        wkm = load_w_cols(wk, nc.sync)
        w2_sb = load_w_full(w2, nc.sync)

        # n-chunks of the packed key axis (512 then the remainder)
        nchunks = []
        n0 = 0
        while n0 < vp:
            nsz = min(N512, vp - n0)
            nchunks.append((n0, nsz))
            n0 += nsz

        pTts = {}

        def emit_qk(h):
            pTt = ptp.tile([P, ktv, N512], BF16, tag="pt")
            pTts[h] = pTt
            for t in range((ktv + 1) // 2):
                lt = pslt.tile([P, 2 * N512], F32, tag="lt")
                kts = [kt for kt in (2 * t, 2 * t + 1) if kt < ktv]
                for kt in kts:
                    nc.tensor.matmul(
                        lt[:, (kt % 2) * N512:((kt % 2) + 1) * N512],
                        kTa[0:HD + 1, h, kt * P:(kt + 1) * P],
                        qTa[0:HD + 1, h, :],
                        start=True, stop=True,
                    )
                nc.scalar.activation(
                    out=pTt[:, 2 * t:2 * t + len(kts), :],
                    in_=lt[:, 0:len(kts) * N512].rearrange(
                        "p (t n) -> p t n", n=N512),
                    func=Act.Exp,
                )

        # ---- K projection; QK for heads 0-2 interleaved so the scalar
        # engine's exp stream starts early ----
        for m in range(KT):
            for n0, nsz in nchunks:
                pt = ps.tile([P, N512], F32, tag="mm")
                for k in range(KT):
                    nc.tensor.matmul(
                        pt[:, 0:nsz],
                        wkm[m][:, k, :],
                        kin[:, k, n0:n0 + nsz],
                        start=(k == 0), stop=(k == KT - 1),
                    )
                ns = slice(n0, n0 + nsz)
                nc.vector.tensor_copy(kTa[0:HD, 2 * m, ns], pt[0:HD, 0:nsz])
                nc.vector.tensor_copy(kTa[0:HD, 2 * m + 1, ns], pt[HD:P, 0:nsz])
            if m < 3:
                emit_qk(m)

        # ---- attention: V projection + remaining QK + PV pipeline ----
        def emit_v_chunk(m, n):
            pt = ps.tile([P, N512], F32, tag="mm")
            for k in range(KT):
                nc.tensor.matmul(
                    pt[:, :],
                    vin[:, k, m * P:(m + 1) * P],
                    wv_sb[:, k, n * N512:(n + 1) * N512],
                    start=(k == 0), stop=(k == KT - 1),
                )
            nc.vector.tensor_copy(
                vaug_h[:, m, 8 * n:8 * n + 8, 0:64],
                pt[:, :].rearrange("p (h d) -> p h d", d=HD),
            )

        def emit_pv(h):
            hp = (h % 2) * HD
            hm = h // 2
            pTt = pTts.pop(h)
            pv = pvp.tile([HD + 1, N512], F32, tag="pv")
            for kt in range(ktv):
                nc.tensor.matmul(
                    pv[:, :],
                    vaug[:, kt, h * 65:(h + 1) * 65],
                    pTt[:, kt, :],
                    start=(kt == 0), stop=(kt == ktv - 1),
                )
            rc = smal.tile([1, N512], F32, tag="rc")
            bc = smal.tile([HD, N512], F32, tag="bc")
            # denom >= exp(min valid logit) > 0 (each batch has >=1 valid
            # key position), so the fast reciprocal's denorm/zero edge
            # cases cannot occur
            nc.vector.tensor_copy(rc[0:1, :], pv[HD:HD + 1, :])
            nc.vector.reciprocal_approx_fast(out=rc[0:1, :], in_=rc[0:1, :])
            nc.gpsimd.partition_broadcast(bc[:, :], rc[0:1, :])
            nc.vector.tensor_mul(xT[hp:hp + HD, hm, :], pv[0:HD, :], bc[:, :])

        # V chunks n=0 first (PV_0..7 read the n=0 half of vaug), then the
        # PV pipeline with the remaining QKs and V chunks interleaved.
        # pTt window stays <= 4 tiles. The last PV slots are filled with
        # the head-independent part of the first two P-proj chains.
        for m in range(ktv):
            emit_v_chunk(m, 0)
            if m == 0:
                emit_qk(3)

        ppA = {}

        def emit_pproj_A(m):
            # k=0..6 of P-proj chain m: independent of the last PV evicts
            pt = ps.tile([P, N512], F32, tag="mm")
            ppA[m] = pt
            for k in range(KT - 1):
                nc.tensor.matmul(
                    pt[:, :], wp_sb[:, k, m * P:(m + 1) * P], xT[:, k, :],
                    start=(k == 0), stop=(k == KT - 2),
                )

        def emit_pproj_B(m):
            # k=7 into a spare pslt half; A evicts (+bias) then B adds in
            # (a vector op may read only one PSUM operand)
            bt = pslt.tile([P, 2 * N512], F32, tag="lt")
            nc.tensor.matmul(
                bt[:, 0:N512], wp_sb[:, KT - 1, m * P:(m + 1) * P],
                xT[:, KT - 1, :], start=True, stop=True,
            )
            pt = ppA.pop(m)
            nc.vector.tensor_scalar(
                out=xpT[:, m, :], in0=pt[:, :],
                scalar1=bp_sb[:, m:m + 1], scalar2=None, op0=add,
            )
            nc.vector.tensor_add(xpT[:, m, :], bt[:, 0:N512], xpT[:, m, :])

        for h in range(H):
            emit_pv(h)
            if h + 4 < H:
                emit_qk(h + 4)
            if h < ktv:
                emit_v_chunk(h, 1)
            if h == H - 3:
                emit_pproj_A(0)
            elif h == H - 2:
                emit_pproj_A(1)

        # ---- output projection + MLP ----
        emit_pproj_B(0)
        emit_pproj_B(1)
        for m in range(2, KT):
            pt = ps.tile([P, N512], F32, tag="mm")
            for k in range(KT):
                nc.tensor.matmul(
                    pt[:, :], wp_sb[:, k, m * P:(m + 1) * P], xT[:, k, :],
                    start=(k == 0), stop=(k == KT - 1),
                )
            nc.vector.tensor_scalar(
                out=xpT[:, m, :], in0=pt[:, :],
                scalar1=bp_sb[:, m:m + 1], scalar2=None, op0=add,
            )

        for m in range(KT):
            pt = ps.tile([P, N512], F32, tag="mm")
            for k in range(KT):
                nc.tensor.matmul(
                    pt[:, :], w1_sb[:, k, m * P:(m + 1) * P], xpT[:, k, :],
                    start=(k == 0), stop=(k == KT - 1),
                )
            nc.scalar.activation(
                out=h1T[:, m, :], in_=pt[:, :], func=Act.Gelu,
                bias=b1_sb[:, m:m + 1], scale=1.0,
            )

        for m in range(KT):
            pt = ps.tile([P, N512], F32, tag="mm")
            for k in range(KT):
                nc.tensor.matmul(
                    pt[:, :], w2_sb[:, k, m * P:(m + 1) * P], h1T[:, k, :],
                    start=(k == 0), stop=(k == KT - 1),
                )
            ot = outp.tile([P, N512], F32, tag="o")
            nc.vector.scalar_tensor_tensor(
                out=ot[:, :], in0=pt[:, :], scalar=b2_sb[:, m:m + 1],
                in1=xpT[:, m, :], op0=add, op1=add,
            )
            nc.sync.dma_start(out[m * P:(m + 1) * P, :], ot[:, :])

    nc.compile()
    return nc


_prog_cache = {}


def _get_program(vp):
    if vp not in _prog_cache:
        _prog_cache[vp] = build_program(vp)
    return _prog_cache[vp]


def _repack_w_cols(W):
    # [C, C] -> [m, p, k, j] with [m, p, k, j] = W[k*128+p, m*128+j]
    return np.ascontiguousarray(
        W.reshape(KT, P, KT, P).transpose(2, 1, 0, 3))


def make_in_maps(inputs, vp):
    q = np.asarray(inputs["query"], np.float32)
    k = np.asarray(inputs["key"], np.float32)
    v = np.asarray(inputs["value"], np.float32)
    mask = np.asarray(inputs["mask"])
    Wq = np.asarray(inputs["Wq"], np.float32) * SCALE
    bq = np.asarray(inputs["bq"], np.float32) * SCALE
    Wk = np.asarray(inputs["Wk"], np.float32)
    Wv = np.asarray(inputs["Wv"], np.float32)
    bv = np.asarray(inputs["bv"], np.float32)
    Wp = np.asarray(inputs["Wp"], np.float32)
    bp = np.asarray(inputs["bp"], np.float32)
    W1 = np.asarray(inputs["W1"], np.float32)
    b1 = np.asarray(inputs["b1"], np.float32)
    W2 = np.asarray(inputs["W2"], np.float32)
    b2 = np.asarray(inputs["b2"], np.float32)

    bp_eff = bp + bv @ Wp

    shared = {
        "wq": _repack_w_cols(Wq.astype(NPBF16)),
        "wk": _repack_w_cols(Wk.astype(NPBF16)),
        "wv": np.ascontiguousarray(Wv.astype(NPBF16)),
        "wp": np.ascontiguousarray(Wp.astype(NPBF16)),
        "w1": np.ascontiguousarray(W1.astype(NPBF16)),
        "w2": np.ascontiguousarray(W2.astype(NPBF16)),
    }

    def pack_cols(vec):      # [C] -> [P, KT] with [p, j] = vec[j*128+p]
        return np.asarray(vec, np.float32).reshape(KT, P).T

    base = np.zeros((P, 4, KT), np.float32)
    for i, vec in enumerate((bp_eff, b1, b2, bq)):
        base[:, i, :] = pack_cols(vec)

    combined = (mask[:, :S] != 0) | (mask[:, S:2 * S] != 0)   # [B, S]
    in_maps = []
    kt_b, vt_b, mrow_b = {}, {}, {}
    for b in range(B):
        idx = np.nonzero(combined[b])[0]
        nv = len(idx)
        kp = np.zeros((C, vp), NPBF16)
        vpk = np.zeros((C, vp), NPBF16)
        kp[:, :nv] = k[b][idx, :].T.astype(NPBF16)
        vpk[:, :nv] = v[b][idx, :].T.astype(NPBF16)
        kt_b[b] = np.ascontiguousarray(kp)
        vt_b[b] = np.ascontiguousarray(vpk)
        mrow = np.full(vp, MASK_NEG, NPBF16)
        mrow[:nv] = 0.0
        mrow_b[b] = mrow

    for core in range(NCORES):
        b, qs = divmod(core, 2)
        m = dict(shared)
        m["qt_in"] = np.ascontiguousarray(
            q[b, qs * SQ:(qs + 1) * SQ, :].T.astype(NPBF16))
        m["kt_in"] = kt_b[b]
        m["vt_in"] = vt_b[b]
        m["maskrow"] = mrow_b[b]
        m["bvecs"] = np.ascontiguousarray(base)
        in_maps.append(m)
    return in_maps


def run(inputs, trace=False, trace_cores=None):
    mask = np.asarray(inputs["mask"])
    combined = (mask[:, :S] != 0) | (mask[:, S:2 * S] != 0)
    maxv = int(combined.sum(1).max())
    vp = max(P, -(-maxv // P) * P)       # round up to a multiple of 128
    nc = _get_program(vp)
    in_maps = make_in_maps(inputs, vp)
    res = run_bass_kernel_spmd(
        nc, in_maps, core_ids=list(range(NCORES)),
        trace=trace, trace_cores=trace_cores,
    )
    outfull = np.empty((B, S, C), np.float32)
    for core in range(NCORES):
        b, qs = divmod(core, 2)
        outfull[b, qs * SQ:(qs + 1) * SQ, :] = res.results[core]["out"].T
    return outfull, res


def kernel(**inputs):
    outfull, _ = run(inputs)
    return outfull
